# revision 1
# baseline (speedup 1.0000x reference)
"""Trainium2 Bass kernel for nn_Memory_Module (scatter_memory).

Semantics (per sample b, with x = Loc[b,0], y = Loc[b,1]):
    out_k = mem_k with the feature patch written at row y//d_k, col x//d_k.

Strategy: pure data parallel over the batch (16 samples -> 8 cores, 2 each).
Each core:
  1. bulk-copies its mem buffers to the outputs with large contiguous
     DRAM->DRAM DMAs on the SP HWDGE ring (largest level first),
  2. loads the per-sample (row, col) offsets (precomputed on host from Loc)
     into ACT-engine registers,
  3. overwrites each patch region with a dynamic-offset DRAM->DRAM DMA on
     the ACT HWDGE ring, gated per-pair on the matching bulk copy's
     semaphore so patch bytes interleave with the remaining copies.
"""

import contextlib

import numpy as np

import concourse.bass as bass
import concourse.mybir as mybir
from concourse.bass_utils import run_bass_kernel_spmd

BATCH = 16
NCORES = 8
SPC = BATCH // NCORES  # samples per core

# (name, channels, mem height/width, patch height/width, Loc divisor)
LEVELS = (
    ("i1", 64, 256, 64, 2),
    ("i2", 64, 128, 32, 4),
    ("i3", 128, 64, 16, 8),
    ("b", 256, 32, 8, 16),
)

# reference output order: (out_b, out_3, out_2, out_1)
OUTPUT_ORDER = ("b", "i3", "i2", "i1")

# input tensor names keyed by level name
FEAT_INPUT = {
    "b": "bottleneck",
    "i3": "intermediate_3",
    "i2": "intermediate_2",
    "i1": "intermediate_1",
}
MEM_INPUT = {
    "b": "mem_bottleneck",
    "i3": "mem_intermediate_3",
    "i2": "mem_intermediate_2",
    "i1": "mem_intermediate_1",
}


def build_nc(levels=LEVELS, spc=SPC):
    nc = bass.Bass()
    feats, mems, outs = {}, {}, {}
    for nm, C, H, P, _dv in levels:
        feats[nm] = nc.dram_tensor(
            f"feat_{nm}", [spc, C, P, P], mybir.dt.float32, kind="ExternalInput"
        )
        mems[nm] = nc.dram_tensor(
            f"mem_{nm}", [spc, C, H, H], mybir.dt.float32, kind="ExternalInput"
        )
        outs[nm] = nc.dram_tensor(
            f"out_{nm}", [spc, C, H, H], mybir.dt.float32, kind="ExternalOutput"
        )
    nlv = len(levels)
    offs = nc.dram_tensor("offs", [1, spc * 2 * nlv], mybir.dt.int32, kind="ExternalInput")

    with contextlib.ExitStack() as stack:
        offs_sb = stack.enter_context(nc.sbuf_tensor([1, spc * 2 * nlv], mybir.dt.int32))
        osem = stack.enter_context(nc.semaphore("osem"))
        dsem = stack.enter_context(nc.semaphore("dsem"))
        csems = {}
        for nm, _C, _H, _P, _dv in levels:
            for s in range(spc):
                csems[(nm, s)] = stack.enter_context(nc.semaphore(f"c_{nm}_{s}"))
        block = stack.enter_context(nc.Block())

        @block.sync
        def _(sync):
            # Bulk copies, largest level first so its patch can start early.
            for nm, C, H, P, _dv in levels:
                for s in range(spc):
                    sync.dma_start(outs[nm][s], mems[nm][s]).then_inc(
                        csems[(nm, s)], 16
                    )

        @block.scalar
        def _(scalar):
            # Offset load + patch writes on the ACT HWDGE ring so their
            # descriptors don't queue behind the bulk copies' ring.
            scalar.dma_start(offs_sb[:], offs[:]).then_inc(osem, 16)
            scalar.wait_ge(osem, 16)
            vals = {}
            for s in range(spc):
                for li, (nm, _C, H, P, _dv) in enumerate(levels):
                    col = (s * nlv + li) * 2
                    hv = nc.values_load(
                        offs_sb[0:1, col : col + 1],
                        engines=(mybir.EngineType.Activation,),
                        min_val=0,
                        max_val=H - P,
                        skip_runtime_bounds_check=True,
                    )
                    wv = nc.values_load(
                        offs_sb[0:1, col + 1 : col + 2],
                        engines=(mybir.EngineType.Activation,),
                        min_val=0,
                        max_val=H - P,
                        skip_runtime_bounds_check=True,
                    )
                    vals[(nm, s)] = (hv, wv)
            ndone = 0
            for nm, _C, H, P, _dv in levels:
                for s in range(spc):
                    hv, wv = vals[(nm, s)]
                    scalar.wait_ge(csems[(nm, s)], 16)
                    scalar.dma_start(
                        outs[nm][s][:, bass.ds(hv, P), bass.ds(wv, P)],
                        feats[nm][s],
                    ).then_inc(dsem, 16)
                    ndone += 16
            scalar.wait_ge(dsem, ndone)

    return nc


_NC_CACHE = None


def _get_nc():
    global _NC_CACHE
    if _NC_CACHE is None:
        _NC_CACHE = build_nc()
    return _NC_CACHE


def prepare_in_maps(inputs):
    """Shard full inputs into per-core input maps (batch split, 2 per core)."""
    Loc = np.asarray(inputs["Loc"])
    in_maps = []
    for c in range(NCORES):
        sl = slice(c * SPC, (c + 1) * SPC)
        m = {}
        offs = np.zeros((1, SPC * 2 * len(LEVELS)), np.int32)
        for s in range(SPC):
            b = c * SPC + s
            x = int(Loc[b, 0])
            y = int(Loc[b, 1])
            for li, (nm, _C, H, P, dv) in enumerate(LEVELS):
                # lax.dynamic_update_slice clamps offsets so the patch fits
                h = max(0, min(y // dv, H - P))
                w = max(0, min(x // dv, H - P))
                col = (s * len(LEVELS) + li) * 2
                offs[0, col] = h
                offs[0, col + 1] = w
        m["offs"] = offs
        for nm, _C, _H, _P, _dv in LEVELS:
            m[f"feat_{nm}"] = np.ascontiguousarray(
                np.asarray(inputs[FEAT_INPUT[nm]], dtype=np.float32)[sl]
            )
            m[f"mem_{nm}"] = np.ascontiguousarray(
                np.asarray(inputs[MEM_INPUT[nm]], dtype=np.float32)[sl]
            )
        in_maps.append(m)
    return in_maps


def gather_outputs(results):
    return tuple(
        np.concatenate([results[c][f"out_{nm}"] for c in range(NCORES)], axis=0)
        for nm in OUTPUT_ORDER
    )


def kernel(**inputs):
    nc = _get_nc()
    in_maps = prepare_in_maps(inputs)
    res = run_bass_kernel_spmd(nc, in_maps, list(range(NCORES)))
    return gather_outputs(res.results)


# revision 8
# speedup vs baseline: 164.1193x; 164.1193x over previous
"""Trainium2 Bass kernel for nn_Memory_Module (scatter_memory).

Semantics (per sample b, with x = Loc[b,0], y = Loc[b,1]):
    out_k = mem_k with the feature patch written at row y//d_k, col x//d_k.

Strategy: pure data parallel over the batch (16 samples -> 8 cores, 2 each).
Each core:
  1. bulk-copies its mem buffers to the outputs with large contiguous
     DRAM->DRAM DMAs on the SP HWDGE ring (largest level first),
  2. loads the per-sample (row, col) offsets (precomputed on host from Loc)
     into ACT-engine registers,
  3. overwrites each patch region with a dynamic-offset DRAM->DRAM DMA on
     the ACT HWDGE ring, gated per-pair on the matching bulk copy's
     semaphore so patch bytes interleave with the remaining copies.
"""

import contextlib

import numpy as np

import concourse.bass as bass
import concourse.mybir as mybir
from concourse.bass_utils import run_bass_kernel_spmd

BATCH = 16
NCORES = 8
SPC = BATCH // NCORES  # samples per core

# (name, channels, mem height/width, patch height/width, Loc divisor)
LEVELS = (
    ("i1", 64, 256, 64, 2),
    ("i2", 64, 128, 32, 4),
    ("i3", 128, 64, 16, 8),
    ("b", 256, 32, 8, 16),
)

# reference output order: (out_b, out_3, out_2, out_1)
OUTPUT_ORDER = ("b", "i3", "i2", "i1")

# input tensor names keyed by level name
FEAT_INPUT = {
    "b": "bottleneck",
    "i3": "intermediate_3",
    "i2": "intermediate_2",
    "i1": "intermediate_1",
}
MEM_INPUT = {
    "b": "mem_bottleneck",
    "i3": "mem_intermediate_3",
    "i2": "mem_intermediate_2",
    "i1": "mem_intermediate_1",
}


def build_nc(levels=LEVELS, spc=SPC, reps=1, patches=True, patch_single_packet=False,
             xring=False):
    """reps>1 repeats the whole (idempotent) DMA sequence in one NEFF for
    benchmarking: per-rep time = (wall(R2)-wall(R1))/(R2-R1) cancels the
    host/axon dispatch overhead. Output is identical for any reps>=1.
    patches=False drops the patch writes (bench diagnostic only).
    xring=True splits work across both HWDGE rings: sync ring carries
    sample-0 copies + sample-1 patches, scalar ring the mirror image."""
    if xring:
        return _build_nc_xring(levels, spc, reps)
    nc = bass.Bass()
    feats, mems, outs = {}, {}, {}
    for nm, C, H, P, _dv in levels:
        feats[nm] = nc.dram_tensor(
            f"feat_{nm}", [spc, C, P, P], mybir.dt.float32, kind="ExternalInput"
        )
        mems[nm] = nc.dram_tensor(
            f"mem_{nm}", [spc, C, H, H], mybir.dt.float32, kind="ExternalInput"
        )
        outs[nm] = nc.dram_tensor(
            f"out_{nm}", [spc, C, H, H], mybir.dt.float32, kind="ExternalOutput"
        )
    nlv = len(levels)
    offs = nc.dram_tensor("offs", [1, spc * 2 * nlv], mybir.dt.int32, kind="ExternalInput")

    with contextlib.ExitStack() as stack:
        offs_sb = stack.enter_context(nc.sbuf_tensor([1, spc * 2 * nlv], mybir.dt.int32))
        osem = stack.enter_context(nc.semaphore("osem"))
        dsem = stack.enter_context(nc.semaphore("dsem"))
        csems = {}
        for nm, _C, _H, _P, _dv in levels:
            for s in range(spc):
                csems[(nm, s)] = stack.enter_context(nc.semaphore(f"c_{nm}_{s}"))
        block = stack.enter_context(nc.Block())

        @block.sync
        def _(sync):
            # Bulk copies, largest level first so its patch can start early.
            for _r in range(reps):
                for nm, C, H, P, _dv in levels:
                    for s in range(spc):
                        sync.dma_start(outs[nm][s], mems[nm][s]).then_inc(
                            csems[(nm, s)], 16
                        )

        @block.scalar
        def _(scalar):
            if not patches:
                total = 16 * reps * len(levels) * spc
                scalar.wait_ge(
                    list(csems.values())[-1], 16 * reps
                )
                return
            # Offset load + patch writes on the ACT HWDGE ring so their
            # descriptors don't queue behind the bulk copies' ring.
            scalar.dma_start(offs_sb[:], offs[:]).then_inc(osem, 16)
            scalar.wait_ge(osem, 16)
            vals = {}
            for s in range(spc):
                for li, (nm, _C, H, P, _dv) in enumerate(levels):
                    col = (s * nlv + li) * 2
                    hv = nc.values_load(
                        offs_sb[0:1, col : col + 1],
                        engines=(mybir.EngineType.Activation,),
                        min_val=0,
                        max_val=H - P,
                        skip_runtime_bounds_check=True,
                    )
                    wv = nc.values_load(
                        offs_sb[0:1, col + 1 : col + 2],
                        engines=(mybir.EngineType.Activation,),
                        min_val=0,
                        max_val=H - P,
                        skip_runtime_bounds_check=True,
                    )
                    vals[(nm, s)] = (hv, wv)
            ndone = 0
            for r in range(reps):
                for nm, _C, H, P, _dv in levels:
                    for s in range(spc):
                        hv, wv = vals[(nm, s)]
                        scalar.wait_ge(csems[(nm, s)], 16 * (r + 1))
                        scalar.dma_start(
                            outs[nm][s][:, bass.ds(hv, P), bass.ds(wv, P)],
                            feats[nm][s],
                            single_packet=patch_single_packet,
                        ).then_inc(dsem, 16)
                        ndone += 16
            scalar.wait_ge(dsem, ndone)

    return nc


def _build_nc_xring(levels, spc, reps):
    assert spc == 2, "xring split assumes 2 samples per core"
    nc = bass.Bass()
    feats, mems, outs = {}, {}, {}
    for nm, C, H, P, _dv in levels:
        feats[nm] = nc.dram_tensor(
            f"feat_{nm}", [spc, C, P, P], mybir.dt.float32, kind="ExternalInput"
        )
        mems[nm] = nc.dram_tensor(
            f"mem_{nm}", [spc, C, H, H], mybir.dt.float32, kind="ExternalInput"
        )
        outs[nm] = nc.dram_tensor(
            f"out_{nm}", [spc, C, H, H], mybir.dt.float32, kind="ExternalOutput"
        )
    nlv = len(levels)
    offs = nc.dram_tensor("offs", [1, spc * 2 * nlv], mybir.dt.int32, kind="ExternalInput")

    # ring assignment: engine e copies sample e, patches sample 1-e
    ENGS = (mybir.EngineType.SP, mybir.EngineType.Activation)

    with contextlib.ExitStack() as stack:
        offs_sb = stack.enter_context(nc.sbuf_tensor([1, spc * 2 * nlv], mybir.dt.int32))
        osem = stack.enter_context(nc.semaphore("osem"))
        dsem = stack.enter_context(nc.semaphore("dsem"))
        csems = {}
        for nm, _C, _H, _P, _dv in levels:
            for s in range(spc):
                csems[(nm, s)] = stack.enter_context(nc.semaphore(f"c_{nm}_{s}"))
        block = stack.enter_context(nc.Block())

        def emit(eng, eng_type, copy_s, patch_s, load_offs):
            if load_offs:
                eng.dma_start(offs_sb[:], offs[:]).then_inc(osem, 16)
            eng.wait_ge(osem, 16)
            vals = {}
            for li, (nm, _C, H, P, _dv) in enumerate(levels):
                col = (patch_s * nlv + li) * 2
                hv = nc.values_load(
                    offs_sb[0:1, col : col + 1],
                    engines=(eng_type,),
                    min_val=0,
                    max_val=H - P,
                    skip_runtime_bounds_check=True,
                )
                wv = nc.values_load(
                    offs_sb[0:1, col + 1 : col + 2],
                    engines=(eng_type,),
                    min_val=0,
                    max_val=H - P,
                    skip_runtime_bounds_check=True,
                )
                vals[nm] = (hv, wv)
            for r in range(reps):
                for nm, _C, _H, _P, _dv in levels:
                    eng.dma_start(outs[nm][copy_s], mems[nm][copy_s]).then_inc(
                        csems[(nm, copy_s)], 16
                    )
                for nm, _C, H, P, _dv in levels:
                    hv, wv = vals[nm]
                    eng.wait_ge(csems[(nm, patch_s)], 16 * (r + 1))
                    eng.dma_start(
                        outs[nm][patch_s][:, bass.ds(hv, P), bass.ds(wv, P)],
                        feats[nm][patch_s],
                    ).then_inc(dsem, 16)
            eng.wait_ge(dsem, 16 * reps * nlv * spc)

        @block.sync
        def _(sync):
            emit(sync, ENGS[0], copy_s=0, patch_s=1, load_offs=False)

        @block.scalar
        def _(scalar):
            emit(scalar, ENGS[1], copy_s=1, patch_s=0, load_offs=True)

    return nc


_NC_CACHE = None


def _get_nc():
    global _NC_CACHE
    if _NC_CACHE is None:
        _NC_CACHE = build_nc()
    return _NC_CACHE


def prepare_in_maps(inputs):
    """Shard full inputs into per-core input maps (batch split, 2 per core)."""
    Loc = np.asarray(inputs["Loc"])
    in_maps = []
    for c in range(NCORES):
        sl = slice(c * SPC, (c + 1) * SPC)
        m = {}
        offs = np.zeros((1, SPC * 2 * len(LEVELS)), np.int32)
        for s in range(SPC):
            b = c * SPC + s
            x = int(Loc[b, 0])
            y = int(Loc[b, 1])
            for li, (nm, _C, H, P, dv) in enumerate(LEVELS):
                # lax.dynamic_update_slice clamps offsets so the patch fits
                h = max(0, min(y // dv, H - P))
                w = max(0, min(x // dv, H - P))
                col = (s * len(LEVELS) + li) * 2
                offs[0, col] = h
                offs[0, col + 1] = w
        m["offs"] = offs
        for nm, _C, _H, _P, _dv in LEVELS:
            m[f"feat_{nm}"] = np.ascontiguousarray(
                np.asarray(inputs[FEAT_INPUT[nm]], dtype=np.float32)[sl]
            )
            m[f"mem_{nm}"] = np.ascontiguousarray(
                np.asarray(inputs[MEM_INPUT[nm]], dtype=np.float32)[sl]
            )
        in_maps.append(m)
    return in_maps


def gather_outputs(results):
    return tuple(
        np.concatenate([results[c][f"out_{nm}"] for c in range(NCORES)], axis=0)
        for nm in OUTPUT_ORDER
    )


def kernel(**inputs):
    nc = _get_nc()
    in_maps = prepare_in_maps(inputs)
    res = run_bass_kernel_spmd(nc, in_maps, list(range(NCORES)))
    return gather_outputs(res.results)
